# revision 5
# baseline (speedup 1.0000x reference)
"""Trainium2 Bass kernel for a 3-layer CIN (compressed interaction network).

Math (reference):
  x1[b,o,d] = sum_{m,h} x0[b,m,d] x0[b,h,d] k1[m*32+h, o]          (o in 32)
  x2[b,o,d] = sum_{m,h} x0[b,m,d] x1[b,h,d] k2[m*32+h, o]          (o in 128)
  x3[b,o,d] = sum_{m,h} x0[b,m,d] x2[b,h,d] k3[m*128+h, o]         (o in 64)
  logit[b]  = sum_o kf1[o] sum_d x1 + sum kf2 sum_d x2 + sum kf3 sum_d x3

Strategy:
  - Pure data-parallel over batch: 8 cores x 256 batch elements.
  - Layer 3 and the final projection are collapsed algebraically:
      logit3[b] = sum_{m,h,d} G3[m,h] x0[b,m,d] x2[b,h,d],
      G3 = reshape(k3) @ kf3, and since x2 only feeds logit2/logit3 (both
      linear in x2), x2 itself is never materialized: layer 2's matmuls use
      the derived stationary [W2G3 | W2kf2] = k2 @ [G3^T | kf2] producing
      T[m,f] = sum_h G3[m,h] x2[b,h,d] and the logit2 integrand directly.
  - Pairwise tensors Z are built on the vector engine as
      Z_t = rep4(x) * shuffle(X0_sp, mask=t)
    where X0_sp holds x0 rows scattered so a per-quadrant STREAM_SHUFFLE
    with constant mask t yields row 4t+q broadcast across quadrant q.
  - Matmuls run as float32r (full-rate PE) with fp32 accumulation in PSUM.
"""

import numpy as np

import concourse.bass as bass
import concourse.mybir as mybir
import concourse.tile as tile
from concourse import bacc
from concourse.bass_utils import run_bass_kernel_spmd
from concourse.masks import make_identity

N_CORES = 8
B, M, D = 2048, 32, 32
BL = B // N_CORES          # 256 batch elements per core
F = BL * D                 # 8192 free columns per core, f = b*D + d
CH = 512                   # chunk of free columns (= one PSUM bank of fp32)
NCH = F // CH              # 16 chunks
BCH = CH // D              # 16 batch elements per chunk

f32 = mybir.dt.float32
f32r = mybir.dt.float32r

_cache = {}


def _r(ap, dt):
    """bitcast an AP for the matmul dtype."""
    return ap.bitcast(dt)


def _build_program(mm_dt=f32r):
    nc = bacc.Bacc("TRN2", target_bir_lowering=False, debug=False,
                   num_devices=N_CORES)
    x0d = nc.dram_tensor("x0", [BL, M, D], f32, kind="ExternalInput").ap()
    k1d = nc.dram_tensor("k1", [M * M, M], f32, kind="ExternalInput").ap()
    k2d = nc.dram_tensor("k2", [M * M, 128], f32, kind="ExternalInput").ap()
    k3d = nc.dram_tensor("k3", [M * 128, 64], f32, kind="ExternalInput").ap()
    kfd = nc.dram_tensor("kf", [224, 1], f32, kind="ExternalInput").ap()
    yd = nc.dram_tensor("y", [BL, 1], f32, kind="ExternalOutput").ap()

    with tile.TileContext(nc) as tc:
        _emit(tc, nc, x0d, k1d, k2d, k3d, kfd, yd, mm_dt)

    nc.compile()
    return nc


def _emit(tc, nc, x0d, k1d, k2d, k3d, kfd, yd, mm_dt):
    dma = nc.sync.dma_start

    with (
        tc.tile_pool(name="const", bufs=1) as const,
        tc.tile_pool(name="cpool", bufs=12) as cpool,
        tc.tile_pool(name="zpool", bufs=6) as zpool,
        tc.tile_pool(name="rpool", bufs=2) as rpool,
        tc.tile_pool(name="upool", bufs=2) as upool,
    ):
        # ---------- one-time loads ----------
        # x0_rep4[32q+i, b*D+d] = x0[b, i, d]
        x0_rep4 = const.tile([128, F], f32)
        x0_src = x0d.rearrange("b m d -> m b d")
        for q in range(4):
            dma(out=x0_rep4[32 * q:32 * (q + 1), :]
                .rearrange("p (b d) -> p b d", d=D), in_=x0_src)
        # X0_sp[32q+u, b*D+d] = x0[b, 4u+q, d]  (u in 0..7; rest memset)
        x0_spv = x0d.rearrange("b (u q) d -> q u b d", q=4)
        X0_sp = const.tile([128, F], f32)
        nc.gpsimd.memset(X0_sp[:, :], 0.0)
        for q in range(4):
            dma(out=X0_sp[32 * q:32 * q + 8, :]
                .rearrange("p (b d) -> p b d", d=D), in_=x0_spv[q])

        # L1 stationaries: k1s_t[32q+i, o] = k1[i*32 + 4t+q, o]
        k1v = k1d.rearrange("(i s) o -> i s o", s=32)  # [i, 4t+q, o]
        k1s = []
        k1f = const.tile([128, 8, 32], f32)
        for t in range(8):
            for q in range(4):
                dma(out=k1f[32 * q:32 * (q + 1), t, :], in_=k1v[:, 4 * t + q, :])
            kt = const.tile([128, 32], mm_dt, tag=f"k1s{t}")
            nc.vector.tensor_copy(kt[:, :], k1f[:, t, :])
            k1s.append(kt)

        # kf pieces
        kf3_bc = const.tile([128, 64], f32)
        dma(out=kf3_bc[:, :],
            in_=bass.AP(tensor=kfd.tensor, offset=160, ap=[[0, 128], [1, 64]]))
        # final projection weights: rows 0..31 -> U-sums (w=1), row 32 ->
        # logit2 partials (w=1), rows 33..63 zero-pad, rows 64..95 -> x1-sums
        # (w=kf1).  Quadrant-aligned bases only.
        wf_col = const.tile([96, 1], f32)
        nc.vector.memset(wf_col[0:96, :], 0.0)
        nc.vector.memset(wf_col[0:33, :], 1.0)
        dma(out=wf_col[64:96, :], in_=kfd[0:32, :])

        # G3e[h', 0:32] = G3[m, h'] = sum_o k3[m*128+h', o] kf3[o];  col 32 = kf2
        G3e = const.tile([128, 33], f32)
        k3n = const.tile([128, 32, 64], f32)
        dma(out=k3n[:, :, :], in_=k3d.rearrange("(a p) o -> p a o", p=128))
        g3t = const.tile([128, 32, 64], f32)
        kfb = kf3_bc[:, :]
        kf3_b3 = bass.AP(tensor=kfb.tensor, offset=kfb.offset,
                         ap=[kfb.ap[0], [0, 32], kfb.ap[1]])
        nc.vector.tensor_mul(g3t[:, :, :], k3n[:, :, :], kf3_b3)
        nc.vector.tensor_reduce(G3e[:, 0:32], g3t[:, :, :],
                                axis=mybir.AxisListType.X, op=mybir.AluOpType.add)
        dma(out=G3e[:, 32:33], in_=kfd[32:160, :])

        # identity for PE transposes
        id128 = const.tile([128, 128], f32)
        make_identity(nc, id128[:, :])

        # L2 derived stationaries: k2dS_t[p, 0:32] = W2G3[128t+p, :],
        # col 32 = W2kf2[128t+p];  W2G3 = k2 @ G3^T, W2kf2 = k2 @ kf2
        k2dS = []
        with (
            tc.tile_pool(name="prep", bufs=3) as prep,
            tc.tile_pool(name="prep_ps", bufs=2, space="PSUM") as prep_ps,
        ):
            for t in range(8):
                k2n = prep.tile([128, 128], f32, tag="k2n")
                dma(out=k2n[:, :], in_=k2d[128 * t:128 * (t + 1), :])
                k2T_ps = prep_ps.tile([128, 128], f32, tag="tp")
                nc.tensor.transpose(k2T_ps[:, :], k2n[:, :], id128[:, :])
                k2T = prep.tile([128, 128], f32, tag="k2T")
                nc.scalar.copy(k2T[:, :], k2T_ps[:, :])
                w_ps = prep_ps.tile([33, 128], f32, tag="w")
                nc.tensor.matmul(w_ps[:, :], lhsT=G3e[:, :], rhs=k2T[:, :])
                w_sb = prep.tile([33, 128], f32, tag="w_sb")
                nc.scalar.copy(w_sb[:, :], w_ps[:, :])
                kd_ps = prep_ps.tile([128, 33], f32, tag="kd")
                nc.tensor.transpose(kd_ps[:, :], w_sb[:, :], id128[0:33, 0:33])
                kd = const.tile([128, 33], mm_dt, tag=f"k2dS{t}")
                nc.vector.tensor_copy(kd[:, :], kd_ps[:, :])
                k2dS.append(kd)

        # persistent x1 and the staging matrix for the final projection
        x1_pers = const.tile([32, F], f32)
        SL = const.tile([96, BL], f32)
        nc.vector.memset(SL[32:64, :], 0.0)

        with (
            tc.tile_pool(name="ps_x1", bufs=2, space="PSUM") as ps_x1,
            tc.tile_pool(name="ps_T", bufs=2, space="PSUM") as ps_T,
            tc.tile_pool(name="ps_out", bufs=1, space="PSUM") as ps_out,
        ):
            for c in range(NCH):
                sl = slice(c * CH, (c + 1) * CH)
                bsl = slice(c * BCH, (c + 1) * BCH)
                # shared broadcast tiles: C_t[32q+i, f] = x0[b, 4t+q, d]
                cts = []
                for t in range(8):
                    ct = cpool.tile([128, CH], f32, tag="ct")
                    nc.vector.stream_shuffle(ct[:, :], X0_sp[:, sl], [t] * 32)
                    cts.append(ct)
                # ----- layer 1 -----
                x1_ps = ps_x1.tile([32, CH], f32)
                for t in range(8):
                    z = zpool.tile([128, CH], mm_dt, tag="z")
                    nc.vector.tensor_mul(z[:, :], x0_rep4[:, sl], cts[t][:, :])
                    nc.tensor.matmul(x1_ps[:, :], lhsT=k1s[t][:, :],
                                     rhs=z[:, :],
                                     start=(t == 0), stop=(t == 7))
                nc.scalar.copy(x1_pers[:, sl], x1_ps[:, :])
                x1_rep4 = rpool.tile([128, CH], f32)
                for q in range(4):
                    dma(out=x1_rep4[32 * q:32 * (q + 1), :], in_=x1_pers[:, sl])
                # ----- layer 2 (collapsed through G3/kf2) -----
                T_ps = ps_T.tile([33, CH], f32)
                for t in range(8):
                    z = zpool.tile([128, CH], mm_dt, tag="z")
                    nc.vector.tensor_mul(z[:, :], x1_rep4[:, :], cts[t][:, :])
                    nc.tensor.matmul(T_ps[:, :], lhsT=k2dS[t][:, :],
                                     rhs=z[:, :],
                                     start=(t == 0), stop=(t == 7))
                # ----- per-chunk logit partials -----
                u = upool.tile([32, CH], f32)
                nc.vector.tensor_mul(u[:, :], x0_rep4[0:32, sl], T_ps[0:32, :])
                nc.vector.tensor_reduce(SL[0:32, bsl],
                                        u[:, :].rearrange("p (b d) -> p b d", d=D),
                                        axis=mybir.AxisListType.X,
                                        op=mybir.AluOpType.add)
                nc.vector.tensor_reduce(SL[32:33, bsl],
                                        T_ps[32:33, :].rearrange("p (b d) -> p b d", d=D),
                                        axis=mybir.AxisListType.X,
                                        op=mybir.AluOpType.add)
                nc.vector.tensor_reduce(SL[64:96, bsl],
                                        x1_pers[:, sl].rearrange("p (b d) -> p b d", d=D),
                                        axis=mybir.AxisListType.X,
                                        op=mybir.AluOpType.add)

            # ----- final projection: logit[b] = wf . SL[:, b] -----
            logit_ps = ps_out.tile([1, BL], f32)
            nc.tensor.matmul(logit_ps[:, :], lhsT=wf_col[0:96, :], rhs=SL[0:96, :])
            logit_sb = const.tile([1, BL], f32)
            nc.scalar.copy(logit_sb[:, :], logit_ps[:, :])
            dma(out=bass.AP(tensor=yd.tensor, offset=0, ap=[[0, 1], [1, BL]]),
                in_=logit_sb[:, :])


def get_program(mm_dt=f32r):
    key = str(mm_dt)
    if key not in _cache:
        _cache[key] = _build_program(mm_dt)
    return _cache[key]


def kernel(x0, k1, k2, k3, kf):
    x0 = np.ascontiguousarray(np.asarray(x0, dtype=np.float32))
    k1 = np.ascontiguousarray(np.asarray(k1, dtype=np.float32))
    k2 = np.ascontiguousarray(np.asarray(k2, dtype=np.float32))
    k3 = np.ascontiguousarray(np.asarray(k3, dtype=np.float32))
    kf = np.ascontiguousarray(np.asarray(kf, dtype=np.float32))
    nc = get_program()
    in_maps = [
        {"x0": x0[c * BL:(c + 1) * BL], "k1": k1, "k2": k2, "k3": k3, "kf": kf}
        for c in range(N_CORES)
    ]
    res = run_bass_kernel_spmd(nc, in_maps, list(range(N_CORES)))
    return np.concatenate([res.results[c]["y"] for c in range(N_CORES)], axis=0)


# revision 12
# speedup vs baseline: 29.1108x; 29.1108x over previous
"""Trainium2 Bass kernel for a 3-layer CIN (compressed interaction network).

Math (reference):
  x1[b,o,d] = sum_{m,h} x0[b,m,d] x0[b,h,d] k1[m*32+h, o]          (o in 32)
  x2[b,o,d] = sum_{m,h} x0[b,m,d] x1[b,h,d] k2[m*32+h, o]          (o in 128)
  x3[b,o,d] = sum_{m,h} x0[b,m,d] x2[b,h,d] k3[m*128+h, o]         (o in 64)
  logit[b]  = sum_o kf1[o] sum_d x1 + sum kf2 sum_d x2 + sum kf3 sum_d x3

Strategy:
  - Pure data parallel over batch: 8 cores x 256 batch elements.
  - Layer 3 and the final projection are collapsed algebraically:
      logit3[b] = sum_{m,h,d} G3[m,h] x0[b,m,d] x2[b,h,d], G3 = reshape(k3)@kf3.
    Since x2 only feeds logit2/logit3 (both linear in x2), x2 is never
    materialized: layer 2 contracts against the derived stationary
    [W2G3 | W2kf2] = k2 @ [G3^T | kf2], yielding T[m,f] = sum_h G3[m,h] x2
    and the logit2 integrand directly.
  - Pairwise tensors Z are built in bf16 on the vector engine (2x mode) as
      Z_t = rep4(x) * shuffle(X0_sp, mask=t)
    where X0_sp holds x0 rows scattered so a per-quadrant STREAM_SHUFFLE with
    constant mask t yields row 4t+q broadcast across quadrant q.  Shuffles run
    on int32-viewed bf16 pairs (half the elements).  A few multiplies are
    offloaded to GPSIMD to balance engines.
  - Matmuls run in bf16 (full-rate PE) with fp32 accumulation in PSUM.
"""

import numpy as np
import ml_dtypes

import concourse.bass as bass
import concourse.mybir as mybir
import concourse.tile as tile
from concourse import bacc
from concourse.bass_utils import run_bass_kernel_spmd
from concourse.masks import make_identity

N_CORES = 8
B, M, D = 2048, 32, 32
BL = B // N_CORES          # 256 batch elements per core
F = BL * D                 # 8192 bf16 free columns per core, f = b*D + d
F2 = F // 2                # int32-pair columns
CH = 512                   # chunk of free columns (= one fp32 PSUM bank)
CH2 = CH // 2
NCH = F // CH              # 16 chunks
BCH = CH // D              # 16 batch elements per chunk
GP_T = (3, 4)              # Z-mult tile indices offloaded to GPSIMD
SC = 512                  # superchunk bf16 cols (2 PSUM banks)
SC2 = SC // 2              # int32-pair cols per superchunk
NSC = F // SC              # 8 superchunks
BSC = SC // D              # 32 batch elements per superchunk

f32 = mybir.dt.float32
bf16 = mybir.dt.bfloat16
i32 = mybir.dt.int32

_cache = {}


def _build_program(repeat=1):
    nc = bacc.Bacc("TRN2", target_bir_lowering=False, debug=False,
                   num_devices=N_CORES)
    # x0 arrives pre-cast to bf16, packed as int32 pairs along d.
    x0b = nc.dram_tensor("x0b", [BL, M, D // 2], i32, kind="ExternalInput").ap()
    k1d = nc.dram_tensor("k1", [M * M, M], f32, kind="ExternalInput").ap()
    k2d = nc.dram_tensor("k2", [M * M, 128], f32, kind="ExternalInput").ap()
    k3d = nc.dram_tensor("k3", [M * 128, 64], f32, kind="ExternalInput").ap()
    kfd = nc.dram_tensor("kf", [224, 1], f32, kind="ExternalInput").ap()
    yd = nc.dram_tensor("y", [BL, 1], f32, kind="ExternalOutput").ap()

    with tile.TileContext(nc) as tc:
        _emit(tc, nc, x0b, k1d, k2d, k3d, kfd, yd, repeat)

    nc.compile()
    return nc


def _emit(tc, nc, x0b, k1d, k2d, k3d, kfd, yd, repeat=1):
    dma = nc.sync.dma_start

    with (
        tc.tile_pool(name="const", bufs=1) as const,
        tc.tile_pool(name="cpool", bufs=36) as cpool,
        tc.tile_pool(name="xrpool", bufs=4) as xrpool,
        tc.tile_pool(name="xspool", bufs=4) as xspool,
        tc.tile_pool(name="zpool", bufs=12) as zpool,
        tc.tile_pool(name="rpool", bufs=3) as rpool,
        tc.tile_pool(name="vpool", bufs=4) as vpool,
        tc.tile_pool(name="tpool", bufs=2) as tpool,
    ):
        # ---------- one-time loads ----------
        # x0 layout views; actual loads happen per superchunk inside do_l1 so
        # they overlap with compute instead of serializing at kernel start.
        x0_src = x0b.rearrange("b m j -> m b j")              # [m, b, j]
        x0_spv = x0b.rearrange("b (u q) j -> q u b j", q=4)   # [q, u, b, j]

        # L1 stationaries (bf16): k1s_t[32q+i, o] = k1[i*32 + 4t+q, o]
        k1v = k1d.rearrange("(i s) o -> i s o", s=32)  # [i, 4t+q, o]
        k1s = []
        k1f = const.tile([128, 8, 32], f32)
        for t in range(8):
            for q in range(4):
                dma(out=k1f[32 * q:32 * (q + 1), t, :], in_=k1v[:, 4 * t + q, :])
            # columns replicated 4x: the L1 matmul then writes x1 to all 128
            # PSUM partitions directly (no x1 -> rep4 DMA round trip).
            kt = const.tile([128, 128], bf16, tag=f"k1s{t}")
            kin = k1f[:, t, :]
            kin4 = bass.AP(tensor=kin.tensor, offset=kin.offset,
                           ap=[kin.ap[0], [0, 4], kin.ap[1]])
            nc.vector.tensor_copy(kt[:, :].rearrange("p (j o) -> p j o", o=32),
                                  kin4)
            k1s.append(kt)

        # kf pieces
        kf3_bc = const.tile([128, 64], f32)
        dma(out=kf3_bc[:, :],
            in_=bass.AP(tensor=kfd.tensor, offset=160, ap=[[0, 128], [1, 64]]))
        # final projection weights: rows 0..31 -> U-sums (w=1), rows 32..63 ->
        # x1-sums (w=kf1), row 64 -> logit2 partials (w=1).
        wf_col = const.tile([65, 1], f32)
        nc.vector.memset(wf_col[0:65, :], 1.0)
        dma(out=wf_col[32:64, :], in_=kfd[0:32, :])

        # G3e[h', 0:32] = G3[m, h'] = sum_o k3[m*128+h', o] kf3[o]; col 32 = kf2
        G3e = const.tile([128, 33], f32)
        k3n = const.tile([128, 32, 64], f32)
        dma(out=k3n[:, :, :], in_=k3d.rearrange("(a p) o -> p a o", p=128))
        g3t = const.tile([128, 32, 64], f32)
        kfb = kf3_bc[:, :]
        kf3_b3 = bass.AP(tensor=kfb.tensor, offset=kfb.offset,
                         ap=[kfb.ap[0], [0, 32], kfb.ap[1]])
        nc.vector.tensor_mul(g3t[:, :, :], k3n[:, :, :], kf3_b3)
        nc.vector.tensor_reduce(G3e[:, 0:32], g3t[:, :, :],
                                axis=mybir.AxisListType.X, op=mybir.AluOpType.add)
        dma(out=G3e[:, 32:33], in_=kfd[32:160, :])

        # identity for PE transposes
        id128 = const.tile([128, 128], f32)
        make_identity(nc, id128[:, :])

        # L2 derived stationaries (bf16): k2dS_t[p, 0:32] = W2G3[128t+p, :],
        # col 32 = W2kf2[128t+p];  W2G3 = k2 @ G3^T, W2kf2 = k2 @ kf2
        k2dS = []
        with (
            tc.tile_pool(name="prep", bufs=3) as prep,
            tc.tile_pool(name="prep_ps", bufs=2, space="PSUM") as prep_ps,
        ):
            for t in range(8):
                k2n = prep.tile([128, 128], f32, tag="k2n")
                dma(out=k2n[:, :], in_=k2d[128 * t:128 * (t + 1), :])
                k2T_ps = prep_ps.tile([128, 128], f32, tag="tp")
                nc.tensor.transpose(k2T_ps[:, :], k2n[:, :], id128[:, :])
                k2T = prep.tile([128, 128], f32, tag="k2T")
                nc.scalar.copy(k2T[:, :], k2T_ps[:, :])
                w_ps = prep_ps.tile([33, 128], f32, tag="w")
                nc.tensor.matmul(w_ps[:, :], lhsT=G3e[:, :], rhs=k2T[:, :])
                w_sb = prep.tile([33, 128], f32, tag="w_sb")
                nc.scalar.copy(w_sb[:, :], w_ps[:, :])
                kd_ps = prep_ps.tile([128, 33], f32, tag="kd")
                nc.tensor.transpose(kd_ps[:, :], w_sb[:, :], id128[0:33, 0:33])
                kd = const.tile([128, 33], bf16, tag=f"k2dS{t}")
                nc.vector.tensor_copy(kd[:, :], kd_ps[:, :])
                k2dS.append(kd)

        # staging matrix for the final projection
        SL = const.tile([65, BL], f32)

        with (
            tc.tile_pool(name="ps_x1", bufs=3, space="PSUM") as ps_x1,
            tc.tile_pool(name="ps_T", bufs=3, space="PSUM") as ps_T,
        ):
            # Two-stage software pipeline over superchunks (SC bf16 cols = 2
            # PSUM banks): layer 2 of superchunk s-1 is emitted after layer 1
            # of superchunk s, so the in-order PE never stalls on the
            # x1 -> rep4 round trip.  DVE ops span the full superchunk.
            def do_load(s):
                bs = slice(s * BSC, (s + 1) * BSC)
                # x0r4[32q+i, :] = bf16 pairs of x0[b, i, d] for this chunk
                x0r4 = xrpool.tile([128, SC2], i32)
                for q in range(4):
                    dma(out=x0r4[32 * q:32 * (q + 1), :]
                        .rearrange("p (b j) -> p b j", j=D // 2),
                        in_=x0_src[:, bs, :])
                # xsp[32q+u, :] = bf16 pairs of x0[b, 4u+q, d]  (u in 0..7)
                xsp = xspool.tile([128, SC2], i32)
                nc.gpsimd.memset(xsp[:, :], 0)
                for q in range(4):
                    dma(out=xsp[32 * q:32 * q + 8, :]
                        .rearrange("p (b j) -> p b j", j=D // 2),
                        in_=x0_spv[q, :, bs, :])
                return x0r4, xsp

            def do_l1(s, ld):
                x0r4, xsp = ld
                # shared broadcast tiles: C_t[32q+i, f] = x0[b, 4t+q, d]
                cts = []
                for t in range(8):
                    ct = cpool.tile([128, SC2], i32, tag="ct")
                    nc.vector.stream_shuffle(ct[:, :], xsp[:, :], [t] * 32)
                    cts.append(ct)
                x0c = x0r4[:, :].bitcast(bf16)
                # V rows 0..31: U = x0*T, rows 32..63: x1, row 64: logit2
                V = vpool.tile([96, SC], bf16)
                x1_ps = ps_x1.tile([128, SC], f32)
                for t in range(8):
                    z = zpool.tile([128, SC], bf16, tag="z")
                    eng = nc.gpsimd if t in GP_T else nc.vector
                    eng.tensor_mul(z[:, :], x0c, cts[t][:, :].bitcast(bf16))
                    for h in range(SC // CH):
                        hs = slice(h * CH, (h + 1) * CH)
                        nc.tensor.matmul(x1_ps[:, hs], lhsT=k1s[t][:, :],
                                         rhs=z[:, hs],
                                         start=(t == 0), stop=(t == 7))
                x1r = rpool.tile([128, SC], bf16)
                nc.scalar.copy(x1r[:, :], x1_ps[:, :])
                nc.scalar.copy(V[32:64, :], x1_ps[32:64, :])
                return cts, x0c, V, x1r

            def do_l2(s, st):
                cts, x0c, V, x1r = st
                bsl = slice(s * BSC, (s + 1) * BSC)
                T_ps = ps_T.tile([33, SC], f32)
                for t in range(8):
                    z = zpool.tile([128, SC], bf16, tag="z")
                    eng = nc.gpsimd if t in GP_T else nc.vector
                    eng.tensor_mul(z[:, :], x1r[:, :],
                                   cts[t][:, :].bitcast(bf16))
                    for h in range(SC // CH):
                        hs = slice(h * CH, (h + 1) * CH)
                        nc.tensor.matmul(T_ps[:, hs], lhsT=k2dS[t][:, :],
                                         rhs=z[:, hs],
                                         start=(t == 0), stop=(t == 7))
                Tsb = tpool.tile([32, SC], bf16)
                nc.scalar.copy(Tsb[:, :], T_ps[0:32, :])
                nc.scalar.copy(V[64:65, :], T_ps[32:33, :])
                nc.vector.tensor_mul(V[0:32, :], x0c[0:32, :], Tsb[:, :])
                nc.vector.tensor_reduce(SL[0:65, bsl],
                                        V[0:65, :].rearrange("p (b d) -> p b d", d=D),
                                        axis=mybir.AxisListType.X,
                                        op=mybir.AluOpType.add)

            # 3-stage pipeline: load(s+1) || layer1(s) || layer2(s-1).
            # `repeat` > 1 re-runs the whole pipeline for marginal-time
            # measurement on hardware; results are identical each round.
            for _ in range(repeat):
                lds = {0: do_load(0), 1: do_load(1)}
                sts = {}
                sts[0] = do_l1(0, lds.pop(0))
                for s in range(1, NSC):
                    if s + 1 < NSC:
                        lds[s + 1] = do_load(s + 1)
                    sts[s] = do_l1(s, lds.pop(s))
                    do_l2(s - 1, sts.pop(s - 1))
                do_l2(NSC - 1, sts.pop(NSC - 1))

        # ----- final projection: logit[b] = wf . SL[:, b] -----
        with tc.tile_pool(name="ps_out", bufs=1, space="PSUM") as ps_out:
            logit_ps = ps_out.tile([1, BL], f32)
            nc.tensor.matmul(logit_ps[:, :], lhsT=wf_col[0:65, :], rhs=SL[0:65, :])
            logit_sb = const.tile([1, BL], f32)
            nc.scalar.copy(logit_sb[:, :], logit_ps[:, :])
            dma(out=bass.AP(tensor=yd.tensor, offset=0, ap=[[0, 1], [1, BL]]),
                in_=logit_sb[:, :])


def get_program(repeat=1):
    if repeat not in _cache:
        _cache[repeat] = _build_program(repeat)
    return _cache[repeat]


def pack_x0(x0):
    """fp32 [BL*, M, D] -> bf16 pairs viewed as int32 [BL*, M, D//2]."""
    xb = x0.astype(ml_dtypes.bfloat16)
    return np.ascontiguousarray(xb).view(np.int32)


def kernel(x0, k1, k2, k3, kf):
    x0 = np.ascontiguousarray(np.asarray(x0, dtype=np.float32))
    k1 = np.ascontiguousarray(np.asarray(k1, dtype=np.float32))
    k2 = np.ascontiguousarray(np.asarray(k2, dtype=np.float32))
    k3 = np.ascontiguousarray(np.asarray(k3, dtype=np.float32))
    kf = np.ascontiguousarray(np.asarray(kf, dtype=np.float32))
    nc = get_program()
    x0p = pack_x0(x0)
    in_maps = [
        {"x0b": x0p[c * BL:(c + 1) * BL], "k1": k1, "k2": k2, "k3": k3, "kf": kf}
        for c in range(N_CORES)
    ]
    res = run_bass_kernel_spmd(nc, in_maps, list(range(N_CORES)))
    return np.concatenate([res.results[c]["y"] for c in range(N_CORES)], axis=0)
